# revision 46
# baseline (speedup 1.0000x reference)
"""Trainium2 Bass kernel for multi-head attention (nn_Attention_54984171323822).

Reference computation (fp32):
    qkv = x @ w_qkv.T + b_qkv            # [B, N, 3*1024]
    q, k, v -> 16 heads x 64
    attn = softmax(q k^T / 8) v          # per head
    out = attn_flat @ w_out.T + b_out    # [B, N, 1024]

Shapes: B=4, N=2048, HIDDEN=1024, 16 heads x 64.

Sharding (8 NeuronCores): DP=4 over batch x TP=2 over heads. Core c handles
batch c//2 and heads (c%2)*8..(c%2)*8+8. No device collectives: each core
emits a partial output-projection [2048, 1024]; the host sums the TP pairs
and adds b_out (linear, so it commutes).

Differences vs the first working version (503us):
  * Scores for a HEAD PAIR are computed by two concurrent row-group
    matmuls (K=64, M=128): head A streams through PE rows 0-63, head B
    through rows 64-127.  The stream port (128 rows/cycle) is then fully
    used -- 2x on the score phase.
  * softmax exp is split across ScalarE and VectorE.  The ScalarE half is
    activation(Exp, scale=ln2, bias=ln A).  The VectorE half is a
    two-instruction fast exp2: a stock tensor_scalar builds the Schraudolph
    seed bits int32(y*2^23 + 127*2^23) (= 2^floor(y)*(1+frac(y)) as fp32
    bits), then a custom DVE op applies a quadratic mantissa correction
    p(v) = 1 + c1 v + c2 v^2 (v = 1+frac extracted by mask/or).  Both halves
    produce A*2^y (A~0.687); the constant cancels in softmax.  Q weights are
    pre-scaled by log2(e)/8 on the host so scores arrive in the log2 domain.
  * The whole attention pipeline is query-block (512) granular with a
    3-deep score-bank rotation, so exp latency stays off the PE critical
    path and normalization drains overlap the next block's scores.
"""

import math
import sys

sys.path.insert(0, "/opt/trn_rl_repo")

import numpy as np
import ml_dtypes

import concourse.bass as bass
import concourse.bacc as bacc
import concourse.tile as tile
from concourse import mybir
from concourse import bass_utils
from concourse import dve_ops
from concourse.dve_spec import Spec, Src0, C0, C1, C2, One, Bin, AluOp, lower

N_CORES = 8
B = 4
N = 2048
HIDDEN = 1024
N_HEADS = 16
HEAD_DIM = 64
HPC = N_HEADS // 2          # heads per core (TP=2)
EC = HPC * HEAD_DIM         # 512 attention dims per core
TC = N // 128               # 16 token chunks
DC = HIDDEN // 128          # 8 hidden chunks
PAIRS = HPC // 2            # 4 head pairs per core
SCALE = HEAD_DIM ** -0.5
LOG2E = 1.4426950408889634
LN2 = 0.6931471805599453
QSCALE = SCALE * LOG2E      # folded into W_q/b_q on the host

# fast-exp2 constants (see module docstring)
EXP2_C1 = -0.47507681
EXP2_C2 = 0.15998377
EXP2_A = 0.6873410053039131
ACT_BIAS = math.log(EXP2_A)
MANT_MASK = 0x007FFFFF
SEED_SCALE = float(2.0 ** 23)
SEED_BIAS = float(127 * 2 ** 23)

# pipeline knobs
PV_LAG = 3
DVE_KC = frozenset((2, 6, 10, 14))   # kc chunks exp'd on VectorE (rest ScalarE)

BF16 = mybir.dt.bfloat16
F32 = mybir.dt.float32
I32 = mybir.dt.int32
NP_BF16 = ml_dtypes.bfloat16


# --------------------------------------------------------------------------
# custom DVE op: EXP2 mantissa fixup
# --------------------------------------------------------------------------
_m = Bin(AluOp.BITWISE_AND, Src0, C0)
_v = Bin(AluOp.BITWISE_OR, _m, One)
_EXP2_BODY = Src0 * (One + _v * (C1 + _v * C2))


def _ref_exp2fix(in0, in1, s0, s1, imm2):
    g = np.asarray(in0, np.float32)
    mask = np.asarray(s0, np.float32).reshape(-1)[0:1].view(np.int32)[0]
    one = np.float32(1.0).view(np.int32)
    v = ((g.view(np.int32) & mask) | one).view(np.float32)
    return (g * (1 + v * (np.float32(s1) + v * np.float32(imm2)))).astype(np.float32)


def _register_exp2_op():
    name = "EXP2_FIXUP_ANT"
    for op in dve_ops.OPS:
        if op.name == name:
            return op
    spec = Spec(body=_EXP2_BODY, reference=_ref_exp2fix)
    shas = {}
    for ver in ("v3", "v4"):
        s = dve_ops.DveOpSpec(name=name, opcode=0, uops=lower(spec, ver=ver),
                              rd1_en=False)
        shas[ver] = s.sha(ver)
    op = dve_ops.DveOp(name, spec, subdim=False, uops_sha=shas)
    dve_ops.OPS.append(op)
    dve_ops.CUSTOM_DVE_SPECS[name] = op.spec
    dve_ops._SUB_OPCODE_FOR_NAME[name] = (
        dve_ops._CUSTOM_DVE_ROW_BASE + len(dve_ops.OPS) - 1)
    return op


EXP2_FIXUP_ANT = _register_exp2_op()


def _build_kernel_body(nc, tc_ctx, ios):
    import contextlib

    xT, wqkvT, bias_qk, bias_v, w_outT, out = ios
    tc = tc_ctx
    ctx = contextlib.ExitStack()
    with ctx:
        const = ctx.enter_context(tc.tile_pool(name="const", bufs=1))
        work = ctx.enter_context(tc.tile_pool(name="work", bufs=3))
        etp = ctx.enter_context(tc.tile_pool(name="etp", bufs=8))
        gp = ctx.enter_context(tc.tile_pool(name="gp", bufs=4))
        small = ctx.enter_context(tc.tile_pool(name="small", bufs=3))
        accp = ctx.enter_context(tc.tile_pool(name="accp", bufs=4, space="PSUM"))
        stp = ctx.enter_context(tc.tile_pool(name="stp", bufs=2, space="PSUM"))

        # ---- resident SBUF tensors ----
        xT_src = xT.ap().rearrange("(c p) t -> c p t", p=128)
        wq_src = wqkvT.ap().rearrange("(c p) e -> c p e", p=128)
        xT_c = []
        wq_c = []
        for dc in range(DC):
            wt = const.tile([128, 3 * EC], BF16, name=f"wq{dc}", tag=f"wq{dc}")
            wq_c.append(wt)
            xt = const.tile([128, N], BF16, name=f"xc{dc}", tag=f"xc{dc}")
            xT_c.append(xt)
        for dc in range(DC):
            nc.sync.dma_start(out=xT_c[dc][:], in_=xT_src[dc])
        # weight columns in consumption order: V (phase 1b) and pair-0 Q/K
        # slices first, the rest afterwards
        for dc in range(DC):
            nc.scalar.dma_start(out=wq_c[dc][:, 2 * EC:3 * EC],
                                in_=wq_src[dc][:, 2 * EC:3 * EC])
        for dc in range(DC):
            nc.scalar.dma_start(out=wq_c[dc][:, 0:128],
                                in_=wq_src[dc][:, 0:128])
            nc.scalar.dma_start(out=wq_c[dc][:, EC:EC + 128],
                                in_=wq_src[dc][:, EC:EC + 128])
        for dc in range(DC):
            nc.scalar.dma_start(out=wq_c[dc][:, 128:EC],
                                in_=wq_src[dc][:, 128:EC])
            nc.scalar.dma_start(out=wq_c[dc][:, EC + 128:2 * EC],
                                in_=wq_src[dc][:, EC + 128:2 * EC])
        wo_sb = const.tile([128, EC // 128, HIDDEN], BF16, name="wo_sb", tag="wo_sb")
        nc.sync.dma_start(out=wo_sb[:], in_=w_outT.ap().rearrange("(c p) e -> p c e", p=128))
        bqk_sb = const.tile([128, 8], F32, name="bqk_sb", tag="bqk_sb")
        nc.sync.dma_start(out=bqk_sb[:], in_=bias_qk.ap())
        # bias_v broadcast to all partitions ([1, 520] dram, partition step 0)
        bv_sb = const.tile([128, HPC * 65], BF16, name="bv_sb", tag="bv_sb")
        bv_ap = bias_v.ap()
        bv_bcast = bass.AP(tensor=bv_ap.tensor, offset=bv_ap.offset,
                           ap=[[0, 128], [1, HPC * 65]])
        nc.gpsimd.dma_start(out=bv_sb[:], in_=bv_bcast)

        qkT = const.tile([128, 2 * EC // 128, N], BF16, name="qkT", tag="qkT")   # [128, 8, 2048]
        vpp = const.tile([128, TC, HPC * 65], BF16, name="vpp", tag="vpp")       # V'' tiles
        attnT_c = [const.tile([128, N], BF16, name=f"attnT{i}", tag=f"attnT{i}")
                   for i in range(EC // 128)]

        # mask constant for the exp2 fixup (raw int bits in SBUF)
        mask_sb = const.tile([128, 1], I32, name="mask_sb", tag="mask_sb")
        nc.vector.memset(mask_sb[:], MANT_MASK)
        # ScalarE exp bias (ln A) as a [P,1] AP
        abias_sb = const.tile([128, 1], F32, name="abias_sb", tag="abias_sb")
        nc.vector.memset(abias_sb[:], ACT_BIAS)

        # ones columns of V'' (col h*65+64 of every token chunk)
        ones_ap = vpp[:].rearrange("p t (h u) -> p t h u", u=65)[:, :, :, 64:65]
        nc.vector.memset(ones_ap, 1.0)

        # ---- phase 1: qkT = w_qk @ x^T + b (e on partitions) ----
        # emitted group-wise so later pairs' Q/K chunks can interleave with
        # the attention stream (fills PE slack while exp engines run).
        def emit_qk_group(ec, ti, pool=None):
            if pool is None:
                ps = accp.tile([128, 512], F32, name="acc", tag="acc")
            else:
                # borrow a score-bank slot (same tag -> same rotation)
                ps = pool.tile([128, 1024], F32, name="st", tag="st")[:, 0:512]
            for dc in range(DC):
                for eh in range(2):   # col-group pair (output rows 0-63/64-127)
                    nc.tensor.matmul(
                        ps[eh * 64:(eh + 1) * 64, :],
                        wq_c[dc][:, ec * 128 + eh * 64:ec * 128 + (eh + 1) * 64],
                        xT_c[dc][:, ti * 512:(ti + 1) * 512],
                        start=(dc == 0), stop=(dc == DC - 1),
                    )
            nc.vector.tensor_scalar_add(
                qkT[:, ec, ti * 512:(ti + 1) * 512], ps[:],
                bqk_sb[:, ec:ec + 1],
            )

        # ---- phase 1b: V (tokens on partitions) + bias, into V'' layout ----
        # emitted chunk-wise, interleaved into pair 0's first query block so
        # the exp engines start ~25us earlier
        def emit_v_group(ti):
            ps = accp.tile([128, 512], F32, name="acc", tag="acc")
            for dc in range(DC):
                for th in range(2):   # col-group pair
                    nc.tensor.matmul(
                        ps[th * 64:(th + 1) * 64, :],
                        xT_c[dc][:, ti * 128 + th * 64:ti * 128 + (th + 1) * 64],
                        wq_c[dc][:, 2 * EC:3 * EC],
                        start=(dc == 0), stop=(dc == DC - 1),
                    )
            v_out = vpp[:, ti].rearrange("p (h u) -> p h u", u=65)[:, :, 0:64]
            v_in = ps[:].rearrange("p (h u) -> p h u", u=64)
            v_bias = bv_sb[:].rearrange("p (h u) -> p h u", u=65)[:, :, 0:64]
            nc.vector.tensor_tensor(out=v_out, in0=v_in, in1=v_bias,
                                    op=mybir.AluOpType.add)

        # V'' first (its weight columns land first), then pair 0's Q/K
        for ti in range(TC):
            emit_v_group(ti)
        for ec in (0, PAIRS):
            for ti in range(4):
                emit_qk_group(ec, ti)

        # ---- phase 3 helper: out = attnT^T @ w_outT, one token chunk ----
        out3 = out.ap().rearrange("(t p) e -> t p e", p=128)

        def emit_op_group(ti):
            osb = work.tile([128, HIDDEN], F32, name="osb", tag="osb")
            for e5 in range(2):
                po = accp.tile([128, 512], F32, name="po", tag="acc")
                for acx in range(EC // 128):
                    for th in range(2):   # col-group pair
                        nc.tensor.matmul(
                            po[th * 64:(th + 1) * 64, :],
                            attnT_c[acx][:, ti * 128 + th * 64:ti * 128 + (th + 1) * 64],
                            wo_sb[:, acx, e5 * 512:(e5 + 1) * 512],
                            start=(acx == 0), stop=(acx == EC // 128 - 1),
                        )
                nc.vector.tensor_copy(osb[:, e5 * 512:(e5 + 1) * 512], po[:])
            nc.sync.dma_start(out=out3[ti], in_=osb[:])

        # ---- phase 2: attention, one head PAIR at a time ----
        # qkT chunk p holds head 2p on partitions 0-63 and head 2p+1 on 64-127
        # (queries in chunk p, keys in chunk 4+p).  The NEXT pair's Q/K
        # projection groups are interleaved between query blocks; the LAST
        # pair's blocks instead interleave the output projection of token
        # chunks already fully covered by earlier blocks.
        for p in range(PAIRS):
            kch = PAIRS + p
            if p + 1 < PAIRS:
                # (ec, ti) groups of the next pair, 2 per query block
                nxt = [(ec, ti) for ec in (p + 1, PAIRS + p + 1) for ti in range(4)]
            else:
                nxt = []
            for qb in range(4):                  # query blocks of 512
                qs = qb * 512
                # this block's share of the next pair's Q/K projection groups;
                # emitted at kc 2..5, after the exp pipeline is primed.
                qkv_share = list(nxt[qb * 2:(qb + 1) * 2])
                v_share = []
                op_share = []
                pvA = accp.tile([65, 512], F32, name="pvA", tag="acc")
                pvB = accp.tile([65, 512], F32, name="pvB", tag="acc")

                def emit_pv(et, kc):
                    nc.tensor.matmul(
                        pvA[:], vpp[:, kc, (2 * p) * 65:(2 * p + 1) * 65],
                        et[:, 0:512],
                        start=(kc == 0), stop=(kc == TC - 1))
                    nc.tensor.matmul(
                        pvB[:], vpp[:, kc, (2 * p + 1) * 65:(2 * p + 2) * 65],
                        et[:, 512:1024],
                        start=(kc == 0), stop=(kc == TC - 1))

                pend = []
                for kc in range(TC):
                    st = stp.tile([128, 1024], F32, name="st", tag="st")
                    # two concurrent row-group matmuls: K=64, M=128 each
                    nc.tensor.matmul(
                        st[:, 0:512],
                        qkT[0:64, kch, kc * 128:(kc + 1) * 128],
                        qkT[0:64, p, qs:qs + 512],
                        start=True, stop=True)
                    nc.tensor.matmul(
                        st[:, 512:1024],
                        qkT[64:128, kch, kc * 128:(kc + 1) * 128],
                        qkT[64:128, p, qs:qs + 512],
                        start=True, stop=True)
                    et = etp.tile([128, 1024], BF16, name="et", tag="et")
                    if kc in DVE_KC:
                        g = gp.tile([128, 1024], I32, name="g", tag="g")
                        nc.vector.tensor_scalar(
                            out=g[:], in0=st[:],
                            scalar1=SEED_SCALE, scalar2=SEED_BIAS,
                            op0=mybir.AluOpType.mult, op1=mybir.AluOpType.add)
                        nc.vector._custom_dve(
                            EXP2_FIXUP_ANT, out=et[:], in0=g[:].bitcast(F32),
                            s0=mask_sb[:].bitcast(F32),
                            s1=EXP2_C1, imm2=EXP2_C2)
                    else:
                        nc.scalar.activation(
                            out=et[:], in_=st[:],
                            func=mybir.ActivationFunctionType.Exp,
                            scale=LN2, bias=abias_sb[:])
                    pend.append((et, kc))
                    if v_share:
                        emit_v_group(v_share.pop(0))
                    elif kc in (2, 3) and qkv_share:
                        emit_qk_group(*qkv_share.pop(0))
                    elif kc in (2, 3, 4, 5) and op_share:
                        emit_op_group(op_share.pop(0))
                    if len(pend) > PV_LAG:
                        emit_pv(*pend.pop(0))
                for args in pend:
                    emit_pv(*args)

                # normalize this query block: attnT_h = oT' * recip(den)
                for half, pv in ((0, pvA), (1, pvB)):
                    # recip of the whole tile (single-row base-64 APs break the
                    # custom op; rows 0..63 are computed and discarded)
                    rall = small.tile([65, 512], F32, name="rall", tag="rall")
                    nc.vector.reciprocal_approx_fast(out=rall[:], in_=pv[:])
                    den0 = small.tile([1, 512], F32, name="den0", tag="den0")
                    nc.sync.dma_start(out=den0[:], in_=rall[64:65, :])
                    rec = small.tile([64, 512], F32, name="rec", tag="rec")
                    nc.gpsimd.partition_broadcast(rec[:], den0[:], channels=64)
                    if half == 0:
                        nc.vector.tensor_tensor(
                            out=attnT_c[p][0:64, qs:qs + 512],
                            in0=pv[0:64, :], in1=rec[:],
                            op=mybir.AluOpType.mult)
                    else:
                        todd = small.tile([64, 512], BF16, name="todd", tag="todd")
                        nc.vector.tensor_tensor(
                            out=todd[:], in0=pv[0:64, :], in1=rec[:],
                            op=mybir.AluOpType.mult)
                        nc.sync.dma_start(
                            out=attnT_c[p][64:128, qs:qs + 512], in_=todd[:])

                # any leftover interleaved groups (shouldn't happen)
                for ec, ti in qkv_share:
                    emit_qk_group(ec, ti)

        # ---- phase 3: output projection ----
        for ti in range(TC):
            emit_op_group(ti)


def build_nc(num_devices=N_CORES):
    nc = bacc.Bacc("TRN2", target_bir_lowering=False, debug=False,
                   num_devices=num_devices)
    xT = nc.dram_tensor("xT", [HIDDEN, N], BF16, kind="ExternalInput")
    wqkvT = nc.dram_tensor("wqkvT", [HIDDEN, 3 * EC], BF16, kind="ExternalInput")
    bias_qk = nc.dram_tensor("bias_qk", [128, 8], F32, kind="ExternalInput")
    bias_v = nc.dram_tensor("bias_v", [1, HPC * 65], BF16, kind="ExternalInput")
    w_outT = nc.dram_tensor("w_outT", [EC, HIDDEN], BF16, kind="ExternalInput")
    out = nc.dram_tensor("out", [N, HIDDEN], F32, kind="ExternalOutput")
    with tile.TileContext(nc) as tc:
        _build_kernel_body(nc, tc, (xT, wqkvT, bias_qk, bias_v, w_outT, out))
    nc.compile()
    return nc


def make_in_maps(x, w_qkv, b_qkv, w_out):
    """Shard the full inputs into 8 per-core input maps.

    The Q rows of w_qkv/b_qkv are pre-scaled by log2(e)/8 so that on-device
    scores arrive in the log2 domain (see module docstring).
    """
    in_maps = []
    for c in range(N_CORES):
        b = c // 2
        tp = c % 2
        sl = slice(tp * EC, (tp + 1) * EC)
        xT_c = np.ascontiguousarray(x[b].T).astype(NP_BF16)
        wq = w_qkv[sl, :] * QSCALE
        wk = w_qkv[HIDDEN + tp * EC: HIDDEN + (tp + 1) * EC, :]
        wv = w_qkv[2 * HIDDEN + tp * EC: 2 * HIDDEN + (tp + 1) * EC, :]
        wqkvT_c = np.concatenate([wq, wk, wv], axis=0).T.astype(NP_BF16)
        wqkvT_c = np.ascontiguousarray(wqkvT_c)
        bq = b_qkv[tp * EC:(tp + 1) * EC] * QSCALE
        bk = b_qkv[HIDDEN + tp * EC: HIDDEN + (tp + 1) * EC]
        bv = b_qkv[2 * HIDDEN + tp * EC: 2 * HIDDEN + (tp + 1) * EC]
        bias_qk_c = np.concatenate([bq, bk]).reshape(8, 128).T.astype(np.float32)
        bias_qk_c = np.ascontiguousarray(bias_qk_c)
        bias_v_c = np.zeros((1, HPC * 65), np.float32)
        bias_v_c.reshape(HPC, 65)[:, :64] = bv.reshape(HPC, 64)
        bias_v_c = bias_v_c.astype(NP_BF16)
        w_outT_c = np.ascontiguousarray(w_out[:, sl].T).astype(NP_BF16)
        in_maps.append({
            "xT": xT_c,
            "wqkvT": wqkvT_c,
            "bias_qk": bias_qk_c,
            "bias_v": bias_v_c,
            "w_outT": w_outT_c,
        })
    return in_maps


def combine_outputs(results, b_out):
    """results: list of 8 per-core {'out': [N, HIDDEN]} -> full [B, N, HIDDEN]."""
    out = np.empty((B, N, HIDDEN), np.float32)
    for b in range(B):
        out[b] = results[2 * b]["out"] + results[2 * b + 1]["out"]
        out[b] += b_out[None, :].astype(np.float32)
    return out


_NC = None


def _get_nc():
    global _NC
    if _NC is None:
        _NC = build_nc()
    return _NC


def kernel(x, w_qkv, b_qkv, w_out, b_out):
    x = np.asarray(x, np.float32)
    w_qkv = np.asarray(w_qkv, np.float32)
    b_qkv = np.asarray(b_qkv, np.float32)
    w_out = np.asarray(w_out, np.float32)
    b_out = np.asarray(b_out, np.float32)
    nc = _get_nc()
    in_maps = make_in_maps(x, w_qkv, b_qkv, w_out)
    res = bass_utils.run_bass_kernel_spmd(nc, in_maps, core_ids=list(range(N_CORES)))
    return combine_outputs(res.results, b_out)


# revision 47
# speedup vs baseline: 1.1804x; 1.1804x over previous
"""Trainium2 Bass kernel for multi-head attention (nn_Attention_54984171323822).

Reference computation (fp32):
    qkv = x @ w_qkv.T + b_qkv            # [B, N, 3*1024]
    q, k, v -> 16 heads x 64
    attn = softmax(q k^T / 8) v          # per head
    out = attn_flat @ w_out.T + b_out    # [B, N, 1024]

Shapes: B=4, N=2048, HIDDEN=1024, 16 heads x 64.

Sharding (8 NeuronCores): DP=4 over batch x TP=2 over heads. Core c handles
batch c//2 and heads (c%2)*8..(c%2)*8+8. No device collectives: each core
emits a partial output-projection [2048, 1024]; the host sums the TP pairs
and adds b_out (linear, so it commutes).

Differences vs the first working version (503us):
  * Scores for a HEAD PAIR are computed by two concurrent row-group
    matmuls (K=64, M=128): head A streams through PE rows 0-63, head B
    through rows 64-127.  The stream port (128 rows/cycle) is then fully
    used -- 2x on the score phase.
  * softmax exp is split across ScalarE and VectorE.  The ScalarE half is
    activation(Exp, scale=ln2, bias=ln A).  The VectorE half is a
    two-instruction fast exp2: a stock tensor_scalar builds the Schraudolph
    seed bits int32(y*2^23 + 127*2^23) (= 2^floor(y)*(1+frac(y)) as fp32
    bits), then a custom DVE op applies a quadratic mantissa correction
    p(v) = 1 + c1 v + c2 v^2 (v = 1+frac extracted by mask/or).  Both halves
    produce A*2^y (A~0.687); the constant cancels in softmax.  Q weights are
    pre-scaled by log2(e)/8 on the host so scores arrive in the log2 domain.
  * The attention pipeline is query-block (512) granular: double-buffered
    score banks, PV lagging the exp stream by 3 chunks, per-block
    normalization whose drain overlaps the next block's scores, and the
    next pair's QKV-projection groups (plus phase-1b V'' groups) threaded
    into the PE stream so projection work runs while the exp engines grind.
"""

import math
import sys

sys.path.insert(0, "/opt/trn_rl_repo")

import numpy as np
import ml_dtypes

import concourse.bass as bass
import concourse.bacc as bacc
import concourse.tile as tile
from concourse import mybir
from concourse import bass_utils
from concourse import dve_ops
from concourse.dve_spec import Spec, Src0, C0, C1, C2, One, Bin, AluOp, lower

N_CORES = 8
B = 4
N = 2048
HIDDEN = 1024
N_HEADS = 16
HEAD_DIM = 64
HPC = N_HEADS // 2          # heads per core (TP=2)
EC = HPC * HEAD_DIM         # 512 attention dims per core
TC = N // 128               # 16 token chunks
DC = HIDDEN // 128          # 8 hidden chunks
PAIRS = HPC // 2            # 4 head pairs per core
SCALE = HEAD_DIM ** -0.5
LOG2E = 1.4426950408889634
LN2 = 0.6931471805599453
QSCALE = SCALE * LOG2E      # folded into W_q/b_q on the host

# fast-exp2 constants (see module docstring)
EXP2_C1 = -0.47507681
EXP2_C2 = 0.15998377
EXP2_A = 0.6873410053039131
ACT_BIAS = math.log(EXP2_A)
MANT_MASK = 0x007FFFFF
SEED_SCALE = float(2.0 ** 23)
SEED_BIAS = float(127 * 2 ** 23)

# pipeline knobs
PV_LAG = 3
DVE_KC = frozenset((2, 6, 10, 14))   # kc chunks exp'd on VectorE (rest ScalarE)

BF16 = mybir.dt.bfloat16
F32 = mybir.dt.float32
I32 = mybir.dt.int32
NP_BF16 = ml_dtypes.bfloat16


# --------------------------------------------------------------------------
# custom DVE op: EXP2 mantissa fixup
# --------------------------------------------------------------------------
_m = Bin(AluOp.BITWISE_AND, Src0, C0)
_v = Bin(AluOp.BITWISE_OR, _m, One)
_EXP2_BODY = Src0 * (One + _v * (C1 + _v * C2))


def _ref_exp2fix(in0, in1, s0, s1, imm2):
    g = np.asarray(in0, np.float32)
    mask = np.asarray(s0, np.float32).reshape(-1)[0:1].view(np.int32)[0]
    one = np.float32(1.0).view(np.int32)
    v = ((g.view(np.int32) & mask) | one).view(np.float32)
    return (g * (1 + v * (np.float32(s1) + v * np.float32(imm2)))).astype(np.float32)


def _register_exp2_op():
    name = "EXP2_FIXUP_ANT"
    for op in dve_ops.OPS:
        if op.name == name:
            return op
    spec = Spec(body=_EXP2_BODY, reference=_ref_exp2fix)
    shas = {}
    for ver in ("v3", "v4"):
        s = dve_ops.DveOpSpec(name=name, opcode=0, uops=lower(spec, ver=ver),
                              rd1_en=False)
        shas[ver] = s.sha(ver)
    op = dve_ops.DveOp(name, spec, subdim=False, uops_sha=shas)
    dve_ops.OPS.append(op)
    dve_ops.CUSTOM_DVE_SPECS[name] = op.spec
    dve_ops._SUB_OPCODE_FOR_NAME[name] = (
        dve_ops._CUSTOM_DVE_ROW_BASE + len(dve_ops.OPS) - 1)
    return op


EXP2_FIXUP_ANT = _register_exp2_op()


def _build_kernel_body(nc, tc_ctx, ios):
    import contextlib

    xT, wqkvT, bias_qk, bias_v, w_outT, out = ios
    tc = tc_ctx
    ctx = contextlib.ExitStack()
    with ctx:
        const = ctx.enter_context(tc.tile_pool(name="const", bufs=1))
        work = ctx.enter_context(tc.tile_pool(name="work", bufs=3))
        etp = ctx.enter_context(tc.tile_pool(name="etp", bufs=8))
        gp = ctx.enter_context(tc.tile_pool(name="gp", bufs=4))
        small = ctx.enter_context(tc.tile_pool(name="small", bufs=3))
        accp = ctx.enter_context(tc.tile_pool(name="accp", bufs=4, space="PSUM"))
        stp = ctx.enter_context(tc.tile_pool(name="stp", bufs=2, space="PSUM"))

        # ---- resident SBUF tensors ----
        xT_src = xT.ap().rearrange("(c p) t -> c p t", p=128)
        wq_src = wqkvT.ap().rearrange("(c p) e -> c p e", p=128)
        xT_c = []
        wq_c = []
        for dc in range(DC):
            wt = const.tile([128, 3 * EC], BF16, name=f"wq{dc}", tag=f"wq{dc}")
            wq_c.append(wt)
            xt = const.tile([128, N], BF16, name=f"xc{dc}", tag=f"xc{dc}")
            xT_c.append(xt)
        for dc in range(DC):
            nc.sync.dma_start(out=xT_c[dc][:], in_=xT_src[dc])
        # weight columns in consumption order: V (phase 1b) and pair-0 Q/K
        # slices first, the rest afterwards
        for dc in range(DC):
            nc.scalar.dma_start(out=wq_c[dc][:, 2 * EC:3 * EC],
                                in_=wq_src[dc][:, 2 * EC:3 * EC])
        for dc in range(DC):
            nc.scalar.dma_start(out=wq_c[dc][:, 0:128],
                                in_=wq_src[dc][:, 0:128])
            nc.scalar.dma_start(out=wq_c[dc][:, EC:EC + 128],
                                in_=wq_src[dc][:, EC:EC + 128])
        for dc in range(DC):
            nc.scalar.dma_start(out=wq_c[dc][:, 128:EC],
                                in_=wq_src[dc][:, 128:EC])
            nc.scalar.dma_start(out=wq_c[dc][:, EC + 128:2 * EC],
                                in_=wq_src[dc][:, EC + 128:2 * EC])
        wo_sb = const.tile([128, EC // 128, HIDDEN], BF16, name="wo_sb", tag="wo_sb")
        nc.sync.dma_start(out=wo_sb[:], in_=w_outT.ap().rearrange("(c p) e -> p c e", p=128))
        bqk_sb = const.tile([128, 8], F32, name="bqk_sb", tag="bqk_sb")
        nc.sync.dma_start(out=bqk_sb[:], in_=bias_qk.ap())
        # bias_v broadcast to all partitions ([1, 520] dram, partition step 0)
        bv_sb = const.tile([128, HPC * 65], BF16, name="bv_sb", tag="bv_sb")
        bv_ap = bias_v.ap()
        bv_bcast = bass.AP(tensor=bv_ap.tensor, offset=bv_ap.offset,
                           ap=[[0, 128], [1, HPC * 65]])
        nc.gpsimd.dma_start(out=bv_sb[:], in_=bv_bcast)

        qkT = const.tile([128, 2 * EC // 128, N], BF16, name="qkT", tag="qkT")   # [128, 8, 2048]
        vpp = const.tile([128, TC, HPC * 65], BF16, name="vpp", tag="vpp")       # V'' tiles
        attnT_c = [const.tile([128, N], BF16, name=f"attnT{i}", tag=f"attnT{i}")
                   for i in range(EC // 128)]

        # mask constant for the exp2 fixup (raw int bits in SBUF)
        mask_sb = const.tile([128, 1], I32, name="mask_sb", tag="mask_sb")
        nc.vector.memset(mask_sb[:], MANT_MASK)
        # ScalarE exp bias (ln A) as a [P,1] AP
        abias_sb = const.tile([128, 1], F32, name="abias_sb", tag="abias_sb")
        nc.vector.memset(abias_sb[:], ACT_BIAS)

        # ones columns of V'' (col h*65+64 of every token chunk)
        ones_ap = vpp[:].rearrange("p t (h u) -> p t h u", u=65)[:, :, :, 64:65]
        nc.vector.memset(ones_ap, 1.0)

        # ---- phase 1: qkT = w_qk @ x^T + b (e on partitions) ----
        # emitted group-wise so later pairs' Q/K chunks can interleave with
        # the attention stream (fills PE slack while exp engines run).
        def emit_qk_group(ec, ti, pool=None):
            if pool is None:
                ps = accp.tile([128, 512], F32, name="acc", tag="acc")
            else:
                # borrow a score-bank slot (same tag -> same rotation)
                ps = pool.tile([128, 1024], F32, name="st", tag="st")[:, 0:512]
            for dc in range(DC):
                for eh in range(2):   # col-group pair (output rows 0-63/64-127)
                    nc.tensor.matmul(
                        ps[eh * 64:(eh + 1) * 64, :],
                        wq_c[dc][:, ec * 128 + eh * 64:ec * 128 + (eh + 1) * 64],
                        xT_c[dc][:, ti * 512:(ti + 1) * 512],
                        start=(dc == 0), stop=(dc == DC - 1),
                    )
            nc.vector.tensor_scalar_add(
                qkT[:, ec, ti * 512:(ti + 1) * 512], ps[:],
                bqk_sb[:, ec:ec + 1],
            )

        # ---- phase 1b: V (tokens on partitions) + bias, into V'' layout ----
        # emitted chunk-wise, interleaved into pair 0's first query block so
        # the exp engines start ~25us earlier
        def emit_v_group(ti):
            ps = accp.tile([128, 512], F32, name="acc", tag="acc")
            for dc in range(DC):
                for th in range(2):   # col-group pair
                    nc.tensor.matmul(
                        ps[th * 64:(th + 1) * 64, :],
                        xT_c[dc][:, ti * 128 + th * 64:ti * 128 + (th + 1) * 64],
                        wq_c[dc][:, 2 * EC:3 * EC],
                        start=(dc == 0), stop=(dc == DC - 1),
                    )
            v_out = vpp[:, ti].rearrange("p (h u) -> p h u", u=65)[:, :, 0:64]
            v_in = ps[:].rearrange("p (h u) -> p h u", u=64)
            v_bias = bv_sb[:].rearrange("p (h u) -> p h u", u=65)[:, :, 0:64]
            nc.vector.tensor_tensor(out=v_out, in0=v_in, in1=v_bias,
                                    op=mybir.AluOpType.add)

        # V'' first (its weight columns land first), then pair 0's Q/K
        for ti in range(TC):
            emit_v_group(ti)
        for ec in (0, PAIRS):
            for ti in range(4):
                emit_qk_group(ec, ti)

        # ---- phase 3 helper: out = attnT^T @ w_outT, one token chunk ----
        out3 = out.ap().rearrange("(t p) e -> t p e", p=128)

        def emit_op_group(ti):
            osb = work.tile([128, HIDDEN], F32, name="osb", tag="osb")
            for e5 in range(2):
                po = accp.tile([128, 512], F32, name="po", tag="acc")
                for acx in range(EC // 128):
                    for th in range(2):   # col-group pair
                        nc.tensor.matmul(
                            po[th * 64:(th + 1) * 64, :],
                            attnT_c[acx][:, ti * 128 + th * 64:ti * 128 + (th + 1) * 64],
                            wo_sb[:, acx, e5 * 512:(e5 + 1) * 512],
                            start=(acx == 0), stop=(acx == EC // 128 - 1),
                        )
                nc.vector.tensor_copy(osb[:, e5 * 512:(e5 + 1) * 512], po[:])
            nc.sync.dma_start(out=out3[ti], in_=osb[:])

        # ---- phase 2: attention, one head PAIR at a time ----
        # qkT chunk p holds head 2p on partitions 0-63 and head 2p+1 on 64-127
        # (queries in chunk p, keys in chunk 4+p).  The NEXT pair's Q/K
        # projection groups are interleaved between query blocks; the LAST
        # pair's blocks instead interleave the output projection of token
        # chunks already fully covered by earlier blocks.
        for p in range(PAIRS):
            kch = PAIRS + p
            if p + 1 < PAIRS:
                # (ec, ti) groups of the next pair, 2 per query block
                nxt = [(ec, ti) for ec in (p + 1, PAIRS + p + 1) for ti in range(4)]
            else:
                nxt = []
            for qb in range(4):                  # query blocks of 512
                qs = qb * 512
                # this block's share of the next pair's Q/K projection groups;
                # emitted at kc 2..5, after the exp pipeline is primed.
                qkv_share = list(nxt[qb * 2:(qb + 1) * 2])
                v_share = []
                op_share = []
                pvA = accp.tile([65, 512], F32, name="pvA", tag="acc")
                pvB = accp.tile([65, 512], F32, name="pvB", tag="acc")

                def emit_pv(et, kc):
                    nc.tensor.matmul(
                        pvA[:], vpp[:, kc, (2 * p) * 65:(2 * p + 1) * 65],
                        et[:, 0:512],
                        start=(kc == 0), stop=(kc == TC - 1))
                    nc.tensor.matmul(
                        pvB[:], vpp[:, kc, (2 * p + 1) * 65:(2 * p + 2) * 65],
                        et[:, 512:1024],
                        start=(kc == 0), stop=(kc == TC - 1))

                pend = []
                for kc in range(TC):
                    st = stp.tile([128, 1024], F32, name="st", tag="st")
                    # two concurrent row-group matmuls: K=64, M=128 each
                    nc.tensor.matmul(
                        st[:, 0:512],
                        qkT[0:64, kch, kc * 128:(kc + 1) * 128],
                        qkT[0:64, p, qs:qs + 512],
                        start=True, stop=True)
                    nc.tensor.matmul(
                        st[:, 512:1024],
                        qkT[64:128, kch, kc * 128:(kc + 1) * 128],
                        qkT[64:128, p, qs:qs + 512],
                        start=True, stop=True)
                    et = etp.tile([128, 1024], BF16, name="et", tag="et")
                    if kc in DVE_KC:
                        g = gp.tile([128, 1024], I32, name="g", tag="g")
                        nc.vector.tensor_scalar(
                            out=g[:], in0=st[:],
                            scalar1=SEED_SCALE, scalar2=SEED_BIAS,
                            op0=mybir.AluOpType.mult, op1=mybir.AluOpType.add)
                        nc.vector._custom_dve(
                            EXP2_FIXUP_ANT, out=et[:], in0=g[:].bitcast(F32),
                            s0=mask_sb[:].bitcast(F32),
                            s1=EXP2_C1, imm2=EXP2_C2)
                    else:
                        nc.scalar.activation(
                            out=et[:], in_=st[:],
                            func=mybir.ActivationFunctionType.Exp,
                            scale=LN2, bias=abias_sb[:])
                    pend.append((et, kc))
                    if v_share:
                        emit_v_group(v_share.pop(0))
                    elif kc in (2, 3) and qkv_share:
                        emit_qk_group(*qkv_share.pop(0))
                    elif kc in (2, 3, 4, 5) and op_share:
                        emit_op_group(op_share.pop(0))
                    if len(pend) > PV_LAG:
                        emit_pv(*pend.pop(0))
                for args in pend:
                    emit_pv(*args)

                # normalize this query block: attnT_h = oT' * recip(den)
                for half, pv in ((0, pvA), (1, pvB)):
                    # recip of the whole tile (single-row base-64 APs break the
                    # custom op; rows 0..63 are computed and discarded)
                    rall = small.tile([65, 512], F32, name="rall", tag="rall")
                    nc.vector.reciprocal_approx_fast(out=rall[:], in_=pv[:])
                    den0 = small.tile([1, 512], F32, name="den0", tag="den0")
                    nc.sync.dma_start(out=den0[:], in_=rall[64:65, :])
                    rec = small.tile([64, 512], F32, name="rec", tag="rec")
                    nc.gpsimd.partition_broadcast(rec[:], den0[:], channels=64)
                    if half == 0:
                        nc.vector.tensor_tensor(
                            out=attnT_c[p][0:64, qs:qs + 512],
                            in0=pv[0:64, :], in1=rec[:],
                            op=mybir.AluOpType.mult)
                    else:
                        todd = small.tile([64, 512], BF16, name="todd", tag="todd")
                        nc.vector.tensor_tensor(
                            out=todd[:], in0=pv[0:64, :], in1=rec[:],
                            op=mybir.AluOpType.mult)
                        nc.sync.dma_start(
                            out=attnT_c[p][64:128, qs:qs + 512], in_=todd[:])

                # any leftover interleaved groups (shouldn't happen)
                for ec, ti in qkv_share:
                    emit_qk_group(ec, ti)

        # ---- phase 3: output projection ----
        for ti in range(TC):
            emit_op_group(ti)


def build_nc(num_devices=N_CORES):
    nc = bacc.Bacc("TRN2", target_bir_lowering=False, debug=False,
                   num_devices=num_devices)
    xT = nc.dram_tensor("xT", [HIDDEN, N], BF16, kind="ExternalInput")
    wqkvT = nc.dram_tensor("wqkvT", [HIDDEN, 3 * EC], BF16, kind="ExternalInput")
    bias_qk = nc.dram_tensor("bias_qk", [128, 8], F32, kind="ExternalInput")
    bias_v = nc.dram_tensor("bias_v", [1, HPC * 65], BF16, kind="ExternalInput")
    w_outT = nc.dram_tensor("w_outT", [EC, HIDDEN], BF16, kind="ExternalInput")
    out = nc.dram_tensor("out", [N, HIDDEN], F32, kind="ExternalOutput")
    with tile.TileContext(nc) as tc:
        _build_kernel_body(nc, tc, (xT, wqkvT, bias_qk, bias_v, w_outT, out))
    nc.compile()
    return nc


def make_in_maps(x, w_qkv, b_qkv, w_out):
    """Shard the full inputs into 8 per-core input maps.

    The Q rows of w_qkv/b_qkv are pre-scaled by log2(e)/8 so that on-device
    scores arrive in the log2 domain (see module docstring).
    """
    in_maps = []
    for c in range(N_CORES):
        b = c // 2
        tp = c % 2
        sl = slice(tp * EC, (tp + 1) * EC)
        xT_c = np.ascontiguousarray(x[b].T).astype(NP_BF16)
        wq = w_qkv[sl, :] * QSCALE
        wk = w_qkv[HIDDEN + tp * EC: HIDDEN + (tp + 1) * EC, :]
        wv = w_qkv[2 * HIDDEN + tp * EC: 2 * HIDDEN + (tp + 1) * EC, :]
        wqkvT_c = np.concatenate([wq, wk, wv], axis=0).T.astype(NP_BF16)
        wqkvT_c = np.ascontiguousarray(wqkvT_c)
        bq = b_qkv[tp * EC:(tp + 1) * EC] * QSCALE
        bk = b_qkv[HIDDEN + tp * EC: HIDDEN + (tp + 1) * EC]
        bv = b_qkv[2 * HIDDEN + tp * EC: 2 * HIDDEN + (tp + 1) * EC]
        bias_qk_c = np.concatenate([bq, bk]).reshape(8, 128).T.astype(np.float32)
        bias_qk_c = np.ascontiguousarray(bias_qk_c)
        bias_v_c = np.zeros((1, HPC * 65), np.float32)
        bias_v_c.reshape(HPC, 65)[:, :64] = bv.reshape(HPC, 64)
        bias_v_c = bias_v_c.astype(NP_BF16)
        w_outT_c = np.ascontiguousarray(w_out[:, sl].T).astype(NP_BF16)
        in_maps.append({
            "xT": xT_c,
            "wqkvT": wqkvT_c,
            "bias_qk": bias_qk_c,
            "bias_v": bias_v_c,
            "w_outT": w_outT_c,
        })
    return in_maps


def combine_outputs(results, b_out):
    """results: list of 8 per-core {'out': [N, HIDDEN]} -> full [B, N, HIDDEN]."""
    out = np.empty((B, N, HIDDEN), np.float32)
    for b in range(B):
        out[b] = results[2 * b]["out"] + results[2 * b + 1]["out"]
        out[b] += b_out[None, :].astype(np.float32)
    return out


_NC = None


def _get_nc():
    global _NC
    if _NC is None:
        _NC = build_nc()
    return _NC


def kernel(x, w_qkv, b_qkv, w_out, b_out):
    x = np.asarray(x, np.float32)
    w_qkv = np.asarray(w_qkv, np.float32)
    b_qkv = np.asarray(b_qkv, np.float32)
    w_out = np.asarray(w_out, np.float32)
    b_out = np.asarray(b_out, np.float32)
    nc = _get_nc()
    in_maps = make_in_maps(x, w_qkv, b_qkv, w_out)
    res = bass_utils.run_bass_kernel_spmd(nc, in_maps, core_ids=list(range(N_CORES)))
    return combine_outputs(res.results, b_out)


# revision 48
# speedup vs baseline: 1.2068x; 1.0224x over previous
"""Trainium2 Bass kernel for multi-head attention (nn_Attention_54984171323822).

Reference computation (fp32):
    qkv = x @ w_qkv.T + b_qkv            # [B, N, 3*1024]
    q, k, v -> 16 heads x 64
    attn = softmax(q k^T / 8) v          # per head
    out = attn_flat @ w_out.T + b_out    # [B, N, 1024]

Shapes: B=4, N=2048, HIDDEN=1024, 16 heads x 64.

Sharding (8 NeuronCores): DP=4 over batch x TP=2 over heads. Core c handles
batch c//2 and heads (c%2)*8..(c%2)*8+8. No device collectives: each core
emits a partial output-projection [2048, 1024]; the host sums the TP pairs
and adds b_out (linear, so it commutes).

Differences vs the first working version (503us):
  * Scores for a HEAD PAIR are computed by two concurrent row-group
    matmuls (K=64, M=128): head A streams through PE rows 0-63, head B
    through rows 64-127.  The stream port (128 rows/cycle) is then fully
    used -- 2x on the score phase.
  * softmax exp is split across ScalarE and VectorE.  The ScalarE half is
    activation(Exp, scale=ln2, bias=ln A).  The VectorE half is a
    two-instruction fast exp2: a stock tensor_scalar builds the Schraudolph
    seed bits int32(y*2^23 + 127*2^23) (= 2^floor(y)*(1+frac(y)) as fp32
    bits), then a custom DVE op applies a quadratic mantissa correction
    p(v) = 1 + c1 v + c2 v^2 (v = 1+frac extracted by mask/or).  Both halves
    produce A*2^y (A~0.687); the constant cancels in softmax.  Q weights are
    pre-scaled by log2(e)/8 on the host so scores arrive in the log2 domain.
  * The attention pipeline is query-block (512) granular: double-buffered
    score banks, PV lagging the exp stream by 3 chunks, per-block
    normalization whose drain overlaps the next block's scores, and the
    next pair's QKV-projection groups (plus phase-1b V'' groups) threaded
    into the PE stream so projection work runs while the exp engines grind.
"""

import math
import sys

sys.path.insert(0, "/opt/trn_rl_repo")

import numpy as np
import ml_dtypes

import concourse.bass as bass
import concourse.bacc as bacc
import concourse.tile as tile
from concourse import mybir
from concourse import bass_utils
from concourse import dve_ops
from concourse.dve_spec import Spec, Src0, C0, C1, C2, One, Bin, AluOp, lower

N_CORES = 8
B = 4
N = 2048
HIDDEN = 1024
N_HEADS = 16
HEAD_DIM = 64
HPC = N_HEADS // 2          # heads per core (TP=2)
EC = HPC * HEAD_DIM         # 512 attention dims per core
TC = N // 128               # 16 token chunks
DC = HIDDEN // 128          # 8 hidden chunks
PAIRS = HPC // 2            # 4 head pairs per core
SCALE = HEAD_DIM ** -0.5
LOG2E = 1.4426950408889634
LN2 = 0.6931471805599453
QSCALE = SCALE * LOG2E      # folded into W_q/b_q on the host

# fast-exp2 constants (see module docstring)
EXP2_C1 = -0.47507681
EXP2_C2 = 0.15998377
EXP2_A = 0.6873410053039131
ACT_BIAS = math.log(EXP2_A)
MANT_MASK = 0x007FFFFF
SEED_SCALE = float(2.0 ** 23)
SEED_BIAS = float(127 * 2 ** 23)

# pipeline knobs
PV_LAG = 3
DVE_KC = frozenset((2, 6, 10, 14))   # kc chunks exp'd on VectorE (rest ScalarE)

BF16 = mybir.dt.bfloat16
F32 = mybir.dt.float32
I32 = mybir.dt.int32
NP_BF16 = ml_dtypes.bfloat16


# --------------------------------------------------------------------------
# custom DVE op: EXP2 mantissa fixup
# --------------------------------------------------------------------------
_m = Bin(AluOp.BITWISE_AND, Src0, C0)
_v = Bin(AluOp.BITWISE_OR, _m, One)
_EXP2_BODY = Src0 * (One + _v * (C1 + _v * C2))


def _ref_exp2fix(in0, in1, s0, s1, imm2):
    g = np.asarray(in0, np.float32)
    mask = np.asarray(s0, np.float32).reshape(-1)[0:1].view(np.int32)[0]
    one = np.float32(1.0).view(np.int32)
    v = ((g.view(np.int32) & mask) | one).view(np.float32)
    return (g * (1 + v * (np.float32(s1) + v * np.float32(imm2)))).astype(np.float32)


def _register_exp2_op():
    name = "EXP2_FIXUP_ANT"
    for op in dve_ops.OPS:
        if op.name == name:
            return op
    spec = Spec(body=_EXP2_BODY, reference=_ref_exp2fix)
    shas = {}
    for ver in ("v3", "v4"):
        s = dve_ops.DveOpSpec(name=name, opcode=0, uops=lower(spec, ver=ver),
                              rd1_en=False)
        shas[ver] = s.sha(ver)
    op = dve_ops.DveOp(name, spec, subdim=False, uops_sha=shas)
    dve_ops.OPS.append(op)
    dve_ops.CUSTOM_DVE_SPECS[name] = op.spec
    dve_ops._SUB_OPCODE_FOR_NAME[name] = (
        dve_ops._CUSTOM_DVE_ROW_BASE + len(dve_ops.OPS) - 1)
    return op


EXP2_FIXUP_ANT = _register_exp2_op()


def _build_kernel_body(nc, tc_ctx, ios):
    import contextlib

    xT, wqkvT, bias_qk, bias_v, w_outT, out = ios
    tc = tc_ctx
    ctx = contextlib.ExitStack()
    with ctx:
        const = ctx.enter_context(tc.tile_pool(name="const", bufs=1))
        work = ctx.enter_context(tc.tile_pool(name="work", bufs=3))
        etp = ctx.enter_context(tc.tile_pool(name="etp", bufs=8))
        gp = ctx.enter_context(tc.tile_pool(name="gp", bufs=4))
        small = ctx.enter_context(tc.tile_pool(name="small", bufs=3))
        accp = ctx.enter_context(tc.tile_pool(name="accp", bufs=4, space="PSUM"))
        stp = ctx.enter_context(tc.tile_pool(name="stp", bufs=2, space="PSUM"))

        # ---- resident SBUF tensors ----
        xT_src = xT.ap().rearrange("(c p) t -> c p t", p=128)
        wq_src = wqkvT.ap().rearrange("(c p) e -> c p e", p=128)
        xT_c = []
        wq_c = []
        for dc in range(DC):
            wt = const.tile([128, 3 * EC], BF16, name=f"wq{dc}", tag=f"wq{dc}")
            wq_c.append(wt)
            xt = const.tile([128, N], BF16, name=f"xc{dc}", tag=f"xc{dc}")
            xT_c.append(xt)
        # x lands token-quarter-major so V'' groups can chase the quarters
        for tq in range(4):
            for dc in range(DC):
                nc.sync.dma_start(out=xT_c[dc][:, tq * 512:(tq + 1) * 512],
                                  in_=xT_src[dc][:, tq * 512:(tq + 1) * 512])
        # weight columns in consumption order: V (phase 1b) and pair-0 Q/K
        # slices first, the rest afterwards
        for dc in range(DC):
            nc.scalar.dma_start(out=wq_c[dc][:, 2 * EC:3 * EC],
                                in_=wq_src[dc][:, 2 * EC:3 * EC])
        for dc in range(DC):
            nc.scalar.dma_start(out=wq_c[dc][:, 0:128],
                                in_=wq_src[dc][:, 0:128])
            nc.scalar.dma_start(out=wq_c[dc][:, EC:EC + 128],
                                in_=wq_src[dc][:, EC:EC + 128])
        for dc in range(DC):
            nc.scalar.dma_start(out=wq_c[dc][:, 128:EC],
                                in_=wq_src[dc][:, 128:EC])
            nc.scalar.dma_start(out=wq_c[dc][:, EC + 128:2 * EC],
                                in_=wq_src[dc][:, EC + 128:2 * EC])
        wo_sb = const.tile([128, EC // 128, HIDDEN], BF16, name="wo_sb", tag="wo_sb")
        nc.sync.dma_start(out=wo_sb[:], in_=w_outT.ap().rearrange("(c p) e -> p c e", p=128))
        bqk_sb = const.tile([128, 8], F32, name="bqk_sb", tag="bqk_sb")
        nc.sync.dma_start(out=bqk_sb[:], in_=bias_qk.ap())
        # bias_v broadcast to all partitions ([1, 520] dram, partition step 0)
        bv_sb = const.tile([128, HPC * 65], BF16, name="bv_sb", tag="bv_sb")
        bv_ap = bias_v.ap()
        bv_bcast = bass.AP(tensor=bv_ap.tensor, offset=bv_ap.offset,
                           ap=[[0, 128], [1, HPC * 65]])
        nc.gpsimd.dma_start(out=bv_sb[:], in_=bv_bcast)

        qkT = const.tile([128, 2 * EC // 128, N], BF16, name="qkT", tag="qkT")   # [128, 8, 2048]
        vpp = const.tile([128, TC, HPC * 65], BF16, name="vpp", tag="vpp")       # V'' tiles
        attnT_c = [const.tile([128, N], BF16, name=f"attnT{i}", tag=f"attnT{i}")
                   for i in range(EC // 128)]

        # mask constant for the exp2 fixup (raw int bits in SBUF)
        mask_sb = const.tile([128, 1], I32, name="mask_sb", tag="mask_sb")
        nc.vector.memset(mask_sb[:], MANT_MASK)
        # ScalarE exp bias (ln A) as a [P,1] AP
        abias_sb = const.tile([128, 1], F32, name="abias_sb", tag="abias_sb")
        nc.vector.memset(abias_sb[:], ACT_BIAS)

        # ones columns of V'' (col h*65+64 of every token chunk)
        ones_ap = vpp[:].rearrange("p t (h u) -> p t h u", u=65)[:, :, :, 64:65]
        nc.vector.memset(ones_ap, 1.0)

        # ---- phase 1: qkT = w_qk @ x^T + b (e on partitions) ----
        # emitted group-wise so later pairs' Q/K chunks can interleave with
        # the attention stream (fills PE slack while exp engines run).
        def emit_qk_group(ec, ti, pool=None):
            if pool is None:
                ps = accp.tile([128, 512], F32, name="acc", tag="acc")
            else:
                # borrow a score-bank slot (same tag -> same rotation)
                ps = pool.tile([128, 1024], F32, name="st", tag="st")[:, 0:512]
            for dc in range(DC):
                for eh in range(2):   # col-group pair (output rows 0-63/64-127)
                    nc.tensor.matmul(
                        ps[eh * 64:(eh + 1) * 64, :],
                        wq_c[dc][:, ec * 128 + eh * 64:ec * 128 + (eh + 1) * 64],
                        xT_c[dc][:, ti * 512:(ti + 1) * 512],
                        start=(dc == 0), stop=(dc == DC - 1),
                    )
            nc.vector.tensor_scalar_add(
                qkT[:, ec, ti * 512:(ti + 1) * 512], ps[:],
                bqk_sb[:, ec:ec + 1],
            )

        # ---- phase 1b: V (tokens on partitions) + bias, into V'' layout ----
        # emitted chunk-wise, interleaved into pair 0's first query block so
        # the exp engines start ~25us earlier
        def emit_v_group(ti):
            ps = accp.tile([128, 512], F32, name="acc", tag="acc")
            for dc in range(DC):
                for th in range(2):   # col-group pair
                    nc.tensor.matmul(
                        ps[th * 64:(th + 1) * 64, :],
                        xT_c[dc][:, ti * 128 + th * 64:ti * 128 + (th + 1) * 64],
                        wq_c[dc][:, 2 * EC:3 * EC],
                        start=(dc == 0), stop=(dc == DC - 1),
                    )
            v_out = vpp[:, ti].rearrange("p (h u) -> p h u", u=65)[:, :, 0:64]
            v_in = ps[:].rearrange("p (h u) -> p h u", u=64)
            v_bias = bv_sb[:].rearrange("p (h u) -> p h u", u=65)[:, :, 0:64]
            nc.vector.tensor_tensor(out=v_out, in0=v_in, in1=v_bias,
                                    op=mybir.AluOpType.add)

        # V'' first (weight columns and x quarters land first), then pair-0 Q/K
        for ti in range(TC):
            emit_v_group(ti)
        for ec in (0, PAIRS):
            for ti in range(4):
                emit_qk_group(ec, ti)

        # ---- phase 3 helper: out = attnT^T @ w_outT, one token chunk ----
        out3 = out.ap().rearrange("(t p) e -> t p e", p=128)

        def emit_op_group(ti):
            osb = work.tile([128, HIDDEN], F32, name="osb", tag="osb")
            for e5 in range(2):
                po = accp.tile([128, 512], F32, name="po", tag="acc")
                for acx in range(EC // 128):
                    for th in range(2):   # col-group pair
                        nc.tensor.matmul(
                            po[th * 64:(th + 1) * 64, :],
                            attnT_c[acx][:, ti * 128 + th * 64:ti * 128 + (th + 1) * 64],
                            wo_sb[:, acx, e5 * 512:(e5 + 1) * 512],
                            start=(acx == 0), stop=(acx == EC // 128 - 1),
                        )
                nc.vector.tensor_copy(osb[:, e5 * 512:(e5 + 1) * 512], po[:])
            nc.sync.dma_start(out=out3[ti], in_=osb[:])

        # ---- phase 2: attention, one head PAIR at a time ----
        # qkT chunk p holds head 2p on partitions 0-63 and head 2p+1 on 64-127
        # (queries in chunk p, keys in chunk 4+p).  The NEXT pair's Q/K
        # projection groups are interleaved between query blocks; the LAST
        # pair's blocks instead interleave the output projection of token
        # chunks already fully covered by earlier blocks.
        for p in range(PAIRS):
            kch = PAIRS + p
            if p + 1 < PAIRS:
                # (ec, ti) groups of the next pair, 2 per query block
                nxt = [(ec, ti) for ec in (p + 1, PAIRS + p + 1) for ti in range(4)]
            else:
                nxt = []
            for qb in range(4):                  # query blocks of 512
                qs = qb * 512
                # this block's share of the next pair's Q/K projection groups;
                # emitted at kc 2..5, after the exp pipeline is primed.
                qkv_share = list(nxt[qb * 2:(qb + 1) * 2])
                v_share = []
                op_share = []
                pvA = accp.tile([65, 512], F32, name="pvA", tag="acc")
                pvB = accp.tile([65, 512], F32, name="pvB", tag="acc")

                def emit_pv(et, kc):
                    nc.tensor.matmul(
                        pvA[:], vpp[:, kc, (2 * p) * 65:(2 * p + 1) * 65],
                        et[:, 0:512],
                        start=(kc == 0), stop=(kc == TC - 1))
                    nc.tensor.matmul(
                        pvB[:], vpp[:, kc, (2 * p + 1) * 65:(2 * p + 2) * 65],
                        et[:, 512:1024],
                        start=(kc == 0), stop=(kc == TC - 1))

                pend = []
                for kc in range(TC):
                    st = stp.tile([128, 1024], F32, name="st", tag="st")
                    # two concurrent row-group matmuls: K=64, M=128 each
                    nc.tensor.matmul(
                        st[:, 0:512],
                        qkT[0:64, kch, kc * 128:(kc + 1) * 128],
                        qkT[0:64, p, qs:qs + 512],
                        start=True, stop=True)
                    nc.tensor.matmul(
                        st[:, 512:1024],
                        qkT[64:128, kch, kc * 128:(kc + 1) * 128],
                        qkT[64:128, p, qs:qs + 512],
                        start=True, stop=True)
                    et = etp.tile([128, 1024], BF16, name="et", tag="et")
                    if kc in DVE_KC:
                        g = gp.tile([128, 1024], I32, name="g", tag="g")
                        nc.vector.tensor_scalar(
                            out=g[:], in0=st[:],
                            scalar1=SEED_SCALE, scalar2=SEED_BIAS,
                            op0=mybir.AluOpType.mult, op1=mybir.AluOpType.add)
                        nc.vector._custom_dve(
                            EXP2_FIXUP_ANT, out=et[:], in0=g[:].bitcast(F32),
                            s0=mask_sb[:].bitcast(F32),
                            s1=EXP2_C1, imm2=EXP2_C2)
                    else:
                        nc.scalar.activation(
                            out=et[:], in_=st[:],
                            func=mybir.ActivationFunctionType.Exp,
                            scale=LN2, bias=abias_sb[:])
                    pend.append((et, kc))
                    if v_share:
                        emit_v_group(v_share.pop(0))
                    elif kc in (2, 3) and qkv_share:
                        emit_qk_group(*qkv_share.pop(0))
                    elif kc in (2, 3, 4, 5) and op_share:
                        emit_op_group(op_share.pop(0))
                    if len(pend) > PV_LAG:
                        emit_pv(*pend.pop(0))
                for args in pend:
                    emit_pv(*args)

                # normalize this query block: attnT_h = oT' * recip(den)
                for half, pv in ((0, pvA), (1, pvB)):
                    # recip of the whole tile (single-row base-64 APs break the
                    # custom op; rows 0..63 are computed and discarded)
                    rall = small.tile([65, 512], F32, name="rall", tag="rall")
                    nc.vector.reciprocal_approx_fast(out=rall[:], in_=pv[:])
                    den0 = small.tile([1, 512], F32, name="den0", tag="den0")
                    nc.sync.dma_start(out=den0[:], in_=rall[64:65, :])
                    rec = small.tile([64, 512], F32, name="rec", tag="rec")
                    nc.gpsimd.partition_broadcast(rec[:], den0[:], channels=64)
                    if half == 0:
                        nc.vector.tensor_tensor(
                            out=attnT_c[p][0:64, qs:qs + 512],
                            in0=pv[0:64, :], in1=rec[:],
                            op=mybir.AluOpType.mult)
                    else:
                        todd = small.tile([64, 512], BF16, name="todd", tag="todd")
                        nc.vector.tensor_tensor(
                            out=todd[:], in0=pv[0:64, :], in1=rec[:],
                            op=mybir.AluOpType.mult)
                        nc.sync.dma_start(
                            out=attnT_c[p][64:128, qs:qs + 512], in_=todd[:])

                # any leftover interleaved groups (shouldn't happen)
                for ec, ti in qkv_share:
                    emit_qk_group(ec, ti)

        # ---- phase 3: output projection ----
        for ti in range(TC):
            emit_op_group(ti)


def build_nc(num_devices=N_CORES):
    nc = bacc.Bacc("TRN2", target_bir_lowering=False, debug=False,
                   num_devices=num_devices)
    xT = nc.dram_tensor("xT", [HIDDEN, N], BF16, kind="ExternalInput")
    wqkvT = nc.dram_tensor("wqkvT", [HIDDEN, 3 * EC], BF16, kind="ExternalInput")
    bias_qk = nc.dram_tensor("bias_qk", [128, 8], F32, kind="ExternalInput")
    bias_v = nc.dram_tensor("bias_v", [1, HPC * 65], BF16, kind="ExternalInput")
    w_outT = nc.dram_tensor("w_outT", [EC, HIDDEN], BF16, kind="ExternalInput")
    out = nc.dram_tensor("out", [N, HIDDEN], F32, kind="ExternalOutput")
    with tile.TileContext(nc) as tc:
        _build_kernel_body(nc, tc, (xT, wqkvT, bias_qk, bias_v, w_outT, out))
    nc.compile()
    return nc


def make_in_maps(x, w_qkv, b_qkv, w_out):
    """Shard the full inputs into 8 per-core input maps.

    The Q rows of w_qkv/b_qkv are pre-scaled by log2(e)/8 so that on-device
    scores arrive in the log2 domain (see module docstring).
    """
    in_maps = []
    for c in range(N_CORES):
        b = c // 2
        tp = c % 2
        sl = slice(tp * EC, (tp + 1) * EC)
        xT_c = np.ascontiguousarray(x[b].T).astype(NP_BF16)
        wq = w_qkv[sl, :] * QSCALE
        wk = w_qkv[HIDDEN + tp * EC: HIDDEN + (tp + 1) * EC, :]
        wv = w_qkv[2 * HIDDEN + tp * EC: 2 * HIDDEN + (tp + 1) * EC, :]
        wqkvT_c = np.concatenate([wq, wk, wv], axis=0).T.astype(NP_BF16)
        wqkvT_c = np.ascontiguousarray(wqkvT_c)
        bq = b_qkv[tp * EC:(tp + 1) * EC] * QSCALE
        bk = b_qkv[HIDDEN + tp * EC: HIDDEN + (tp + 1) * EC]
        bv = b_qkv[2 * HIDDEN + tp * EC: 2 * HIDDEN + (tp + 1) * EC]
        bias_qk_c = np.concatenate([bq, bk]).reshape(8, 128).T.astype(np.float32)
        bias_qk_c = np.ascontiguousarray(bias_qk_c)
        bias_v_c = np.zeros((1, HPC * 65), np.float32)
        bias_v_c.reshape(HPC, 65)[:, :64] = bv.reshape(HPC, 64)
        bias_v_c = bias_v_c.astype(NP_BF16)
        w_outT_c = np.ascontiguousarray(w_out[:, sl].T).astype(NP_BF16)
        in_maps.append({
            "xT": xT_c,
            "wqkvT": wqkvT_c,
            "bias_qk": bias_qk_c,
            "bias_v": bias_v_c,
            "w_outT": w_outT_c,
        })
    return in_maps


def combine_outputs(results, b_out):
    """results: list of 8 per-core {'out': [N, HIDDEN]} -> full [B, N, HIDDEN]."""
    out = np.empty((B, N, HIDDEN), np.float32)
    for b in range(B):
        out[b] = results[2 * b]["out"] + results[2 * b + 1]["out"]
        out[b] += b_out[None, :].astype(np.float32)
    return out


_NC = None


def _get_nc():
    global _NC
    if _NC is None:
        _NC = build_nc()
    return _NC


def kernel(x, w_qkv, b_qkv, w_out, b_out):
    x = np.asarray(x, np.float32)
    w_qkv = np.asarray(w_qkv, np.float32)
    b_qkv = np.asarray(b_qkv, np.float32)
    w_out = np.asarray(w_out, np.float32)
    b_out = np.asarray(b_out, np.float32)
    nc = _get_nc()
    in_maps = make_in_maps(x, w_qkv, b_qkv, w_out)
    res = bass_utils.run_bass_kernel_spmd(nc, in_maps, core_ids=list(range(N_CORES)))
    return combine_outputs(res.results, b_out)
